# revision 40
# baseline (speedup 1.0000x reference)
"""MoE-routed batched decoder kernel for Trainium2 (8 NeuronCores, SPMD).

Problem: per-sample 2-layer MLP (128 -> 256 relu -> 128) where each sample's
flat weight vector quant_fn[n] is one of 64 codebook rows selected by
emb_idx[n] (VQ semantics: rows are identical within an expert group).

Strategy (expert-parallel MoE routing):
  host: route tokens by emb_idx (stable sort), dedupe weights to the 64
        unique codebook rows (16.9 MB instead of 1.08 GB), pre-transpose
        into PE lhsT layout, pad each expert's token group to P columns;
  device (SPMD x8): core k owns experts [8k, 8k+8); per expert two
        fp32 matmuls for layer 1 (W1^T halves), fused bias+relu, two
        accumulating matmuls for layer 2 (W2^T halves), fused bias add;
  host: inverse-permute per-expert outputs back to token order.
"""

import sys
import types

import numpy as np

for _p in ("/opt/trn_rl_repo", "/root/.axon_site/_ro/trn_rl_repo"):
    if _p not in sys.path:
        sys.path.append(_p)

import concourse.bass as bass
import concourse.tile as tile
from concourse import mybir
from concourse.bass_utils import run_bass_kernel_spmd
from bass_rust import ScopedClock

X_SIZE = 128
H_SIZE = 256
OUT_SIZE = 128
NUM_EMB = 64
N_CORES = 8
EXP_PER_CORE = NUM_EMB // N_CORES
# packed per-expert weight block columns: W1^T [128,256] | W2^T upper [128,128]
# | W2^T lower [128,128] | b1[:128] | b1[128:] | b2 | pad
WCOLS = 516
_I0 = H_SIZE * X_SIZE
_I1 = _I0 + H_SIZE
_I2 = _I1 + OUT_SIZE * H_SIZE

F32 = mybir.dt.float32
F32R = mybir.dt.float32r
MM_F32R = False  # float32r needs explicit rounding ops and loses precision; keep fp32


def _install_axon_shims():
    """Make trace=True survivable in this container: register the ctypes NTFF
    hook for the missing antenv.axon_hooks module, and keep profile artifacts
    local (no bucket creds in the sandbox)."""
    try:
        import antenv

        if "antenv.axon_hooks" not in sys.modules:
            shim = types.ModuleType("antenv.axon_hooks")
            holder = {}
            shim.set_axon_ntff_profile_hook = lambda h: holder.__setitem__("h", h)
            shim.get_axon_ntff_profile_hook = lambda: holder.get("h")
            sys.modules["antenv.axon_hooks"] = shim
            antenv.axon_hooks = shim
            import trn_agent_boot.trn_boot as _tb

            shim.set_axon_ntff_profile_hook(
                _tb._ntff_profile_via_ctypes("/opt/axon/libaxon_pjrt.so")
            )
        import concourse.bass_utils as _bu

        _bu.upload_artifacts = lambda tmpdir: "local://" + tmpdir
    except Exception:
        pass


def _patched_drain_and_barrier(self, tick_clock, wait_clock):
    # The stock implementation piles every outstanding sem wait onto the single
    # tail Drain CTRL, which overflows this walrus build's sync-wait slots.
    # Emit one wait_ge per semaphore on the sync engine, then a bare drain.
    probe = mybir.InstNoOp(
        name="tile_drain_probe", engine=mybir.EngineType.SP, bass_nofuse=True
    )
    wait_clock.add_sem_waits(probe, ScopedClock({None: tick_clock.global_clock}))
    by_name = {h.name: h for h in self.sems.allocated().values()}
    for w in probe.sync_info.on_wait if probe.sync_info else []:
        self.nc.sync.wait_ge(by_name[w.ant_name], w.wait_value)
    self.nc.sync.drain()
    self.nc.all_engine_barrier()
    popped = self.nc._tile_sem_poison_stack.pop()
    assert popped is self._sem_poison
    self.nc.clear_and_free_semaphores(list(self.sems.allocated().values()))
    self.nc.all_engine_barrier()


_orig_lower_ordered_insts = tile.TileContext._lower_ordered_insts
_waitsplit_counter = [0]


def _lower_with_wait_split(self, ordered):
    # This walrus build rejects instructions carrying more than one sync wait
    # ("Too many sync wait commands"). Hoist excess waits into dedicated
    # single-wait NoOps on the same engine, immediately before the offender.
    for bb_name, insts in ordered.items():
        new = []
        for inst in insts:
            si = inst.sync_info
            if si is not None and len(si.on_wait) > 1:
                waits = list(si.on_wait)
                for w in waits[:-1]:
                    _waitsplit_counter[0] += 1
                    new.append(
                        mybir.InstNoOp(
                            name=f"I-waitsplit-{_waitsplit_counter[0]}",
                            engine=inst.engine,
                            sync_info=mybir.SyncInfo(on_wait=[w], on_update=[]),
                            bass_nofuse=True,
                        )
                    )
                inst.sync_info = mybir.SyncInfo(
                    on_wait=[waits[-1]], on_update=list(si.on_update)
                )
            new.append(inst)
        ordered[bb_name] = new
    return _orig_lower_ordered_insts(self, ordered)


tile.TileContext._lower_ordered_insts = _lower_with_wait_split
tile.TileContext._drain_and_barrier = _patched_drain_and_barrier
_install_axon_shims()

_PROGRAM_CACHE: dict[tuple, bass.Bass] = {}
RAW_BASS = True  # hand-scheduled pipeline (no TileContext) — much smaller head/tail


def _build_program_raw(P: int) -> bass.Bass:
    """Static 4-engine pipeline with manual semaphores.

    sync   : x DMA, per-expert weight DMA in, per-expert y DMA out
    tensor : per expert, 4 fp32 matmuls (2x L1 halves, 2x accumulating L2)
    scalar : relu+bias for the first H half; y bias+copy for even experts
    vector : relu+bias for the second H half; y bias+copy for odd experts
    PSUM   : ph_a/ph_b/py double-buffered by expert parity (6 banks)
    """
    E = EXP_PER_CORE
    nc = bass.Bass("TRN2", target_bir_lowering=False, debug=False, num_devices=N_CORES)
    w_d = nc.declare_dram_parameter("w", [E, 128, WCOLS], F32, isOutput=False)
    xt_d = nc.declare_dram_parameter("xt", [128, E * P], F32, isOutput=False)
    yt_d = nc.declare_dram_parameter("yt", [128, E * P], F32, isOutput=True)

    relu = mybir.ActivationFunctionType.Relu
    ident = mybir.ActivationFunctionType.Identity
    add = mybir.AluOpType.add
    amax = mybir.AluOpType.max

    xt = nc.alloc_sbuf_tensor("xt_sb", [128, E * P], F32).ap()
    yt = nc.alloc_sbuf_tensor("yt_sb", [128, E * P], F32).ap()
    w_all = nc.alloc_sbuf_tensor("w_sb", [128, E * WCOLS], F32).ap()
    w_sb = [w_all[:, e * WCOLS : (e + 1) * WCOLS] for e in range(E)]
    scr = nc.alloc_sbuf_tensor("scratch", [128, 512], F32).ap()
    ha = [nc.alloc_sbuf_tensor(f"ha{i}", [128, P], F32).ap() for i in range(2)]
    hb = [nc.alloc_sbuf_tensor(f"hb{i}", [128, P], F32).ap() for i in range(2)]
    pha = [nc.alloc_psum_tensor(f"pha{i}", [128, 512], F32).ap() for i in range(3)]
    phb = [nc.alloc_psum_tensor(f"phb{i}", [128, 512], F32).ap() for i in range(3)]
    ppy = [nc.alloc_psum_tensor(f"ppy{i}", [128, 512], F32).ap() for i in range(2)]
    # DRAM weights viewed [partition, expert, col] for paired-expert DMAs
    w_dp = w_d.ap().rearrange("e p c -> p e c")

    from contextlib import ExitStack

    with ExitStack() as st:
        sem = lambda name: st.enter_context(nc.semaphore(name))
        s_x = sem("s_x")
        s_xr = sem("s_xr")
        s_w = [sem(f"s_w{e}") for e in range(E)]
        s_ph = sem("s_ph")
        s_ha, s_hb = sem("s_ha"), sem("s_hb")
        s_py = sem("s_py")
        s_ye, s_yo = sem("s_ye"), sem("s_yo")
        s_out = sem("s_out")
        block = st.enter_context(nc.Block())

        @block.sync
        def _(sync):
            # per-expert (x slice, weights) bundles in expert order: each expert
            # becomes runnable ~0.77us after the previous one
            sync.dma_start(out=xt[:, : 2 * P], in_=xt_d[:, : 2 * P]).then_inc(s_x, 16)
            sync.dma_start(out=w_sb[0][:], in_=w_dp[:, 0, :]).then_inc(s_w[0], 16)
            sync.dma_start(out=w_sb[1][:], in_=w_dp[:, 1, :]).then_inc(s_w[1], 16)
            for e in range(2, E):
                sync.dma_start(
                    out=xt[:, e * P : (e + 1) * P], in_=xt_d[:, e * P : (e + 1) * P]
                ).then_inc(s_w[e], 16)
                sync.dma_start(
                    out=w_sb[e][:], in_=w_dp[:, e, :]
                ).then_inc(s_w[e], 16)
            for j in range(E // 2):
                # two experts' outputs per DMA (even parity fused, odd standalone)
                sync.wait_ge(s_yo, j + 1)
                dma = sync.dma_start(
                    out=yt_d[:, 2 * j * P : (2 * j + 2) * P],
                    in_=yt[:, 2 * j * P : (2 * j + 2) * P],
                )
                dma._wait_ge(s_ye, j + 1)
                dma.then_inc(s_out, 16)
            sync.wait_ge(s_out, 16 * (E // 2))

        @block.tensor
        def _(tensor):
            # Software-pipelined: L1 of expert e+1 issues before L2 of expert e,
            # so the relu handoff (PE -> ACT/DVE -> PE) hides behind L1 matmuls.
            # Each logical matmul is col-split into two concurrent M=64 tiles
            # (tile_position (0,0)/(0,64)): the 64-col LDWEIGHTS pair overlaps,
            # halving the weight-load bound that dominates fp32 passes.
            def mm_split(out, lhsT, rhs, start, stop, wait=None, inc=None):
                m1 = nc.tensor.matmul(
                    out[0:64, :P], lhsT[:, 0:64], rhs,
                    start=start, stop=stop, tile_position=(0, 0),
                )
                if wait is not None:
                    m1._wait_ge(*wait)
                m2 = nc.tensor.matmul(
                    out[64:128, :P], lhsT[:, 64:128], rhs,
                    start=start, stop=stop, tile_position=(0, 64),
                )
                if inc is not None:
                    m2.then_inc(*inc)

            def l1(e):
                i = e % 3
                w = w_sb[e]
                xe = xt[:, e * P : (e + 1) * P]
                if e >= 3:
                    # ph bank of e-3 free once its relus have read it
                    tensor.wait_ge(s_ha, e - 2)
                mm_split(
                    pha[i], w[:, 0:128], xe, True, True,
                    wait=(s_w[e], 32 if e >= 2 else 16),
                )
                mm_split(
                    phb[i], w[:, 128:256], xe, True, True,
                    wait=(s_hb, e - 2) if e >= 3 else None, inc=(s_ph, 1),
                )

            def l2(e):
                i = e % 2
                w = w_sb[e]
                if e >= 2:
                    # py bank of e-2 free once its y copy completed
                    tensor.wait_ge(s_ye if e % 2 == 0 else s_yo, e // 2)
                mm_split(
                    ppy[i], w[:, 256:384], ha[i][:], True, False, wait=(s_ha, e + 1)
                )
                mm_split(
                    ppy[i], w[:, 384:512], hb[i][:], False, True,
                    wait=(s_hb, e + 1), inc=(s_py, 1),
                )

            # dummy matmuls on scratch data ramp the HAM clock gate to 2.4GHz
            # while the first input DMAs land (~3us of sustained PE activity);
            # they write ppy[0], which l2(0) later resets with start=True
            for _ in range(7):
                nc.tensor.matmul(
                    ppy[0][:, :96], scr[:, :128], scr[:, :96], start=True, stop=True
                )
            tensor.wait_ge(s_x, 16)
            l1(0)
            l1(1)
            l1(2)
            for e in range(E):
                if e + 3 <= E - 1:
                    l1(e + 3)
                l2(e)

        # y-copy ops are emitted two experts late so the relu chain (which gates
        # the PE) never queues behind a y op on the same in-order engine.
        def y_scalar(e):
            y = nc.scalar.activation(
                yt[:, e * P : (e + 1) * P],
                ppy[e % 2][:, :P],
                ident,
                bias=w_sb[e][:, 514:515],
            )
            y._wait_ge(s_py, e + 1)
            y.then_inc(s_ye)

        def y_vector(e):
            y = nc.vector.tensor_scalar(
                yt[:, e * P : (e + 1) * P],
                ppy[e % 2][:, :P],
                w_sb[e][:, 514:515],
                None,
                add,
            )
            y._wait_ge(s_py, e + 1)
            y.then_inc(s_yo)

        @block.scalar
        def _(scalar):
            for e in range(E):
                i = e % 2
                if e >= 2:
                    scalar.wait_ge(s_py, e - 1)  # ha buf consumed by e-2's L2
                act = nc.scalar.activation(
                    ha[i][:], pha[e % 3][:, :P], relu, bias=w_sb[e][:, 512:513]
                )
                act._wait_ge(s_ph, e + 1)
                act.then_inc(s_ha)
                if e >= 2 and (e - 2) % 2 == 0:
                    y_scalar(e - 2)
            y_scalar(E - 2)

        @block.vector
        def _(vector):
            for e in range(E):
                i = e % 2
                if e >= 2:
                    vector.wait_ge(s_py, e - 1)
                ts = nc.vector.tensor_scalar(
                    hb[i][:], phb[e % 3][:, :P], w_sb[e][:, 513:514], 0.0, add, amax
                )
                ts._wait_ge(s_ph, e + 1)
                ts.then_inc(s_hb)
                if e >= 2 and (e - 2) % 2 == 1:
                    y_vector(e - 2)
            y_vector(E - 1)

    return nc


def _build_program(P: int) -> bass.Bass:
    nc = bass.Bass("TRN2", target_bir_lowering=False, debug=False, num_devices=N_CORES)
    w_d = nc.declare_dram_parameter(
        "w", [EXP_PER_CORE, 128, WCOLS], F32, isOutput=False
    )
    xt_d = nc.declare_dram_parameter("xt", [128, EXP_PER_CORE * P], F32, isOutput=False)
    yt_d = nc.declare_dram_parameter("yt", [128, EXP_PER_CORE * P], F32, isOutput=True)

    relu = mybir.ActivationFunctionType.Relu
    ident = mybir.ActivationFunctionType.Identity
    add = mybir.AluOpType.add
    amax = mybir.AluOpType.max

    with tile.TileContext(nc) as tc:
        with (
            tc.tile_pool(name="wpool", bufs=8) as wpool,
            tc.tile_pool(name="xpool", bufs=1) as xpool,
            tc.tile_pool(name="hpool", bufs=6) as hpool,
            tc.tile_pool(name="ypool", bufs=1) as ypool,
            tc.tile_pool(name="psum", bufs=2, space="PSUM") as psum,
        ):
            xt = xpool.tile([128, EXP_PER_CORE * P], F32)
            nc.sync.dma_start(xt[:], xt_d[:])
            yt = ypool.tile([128, EXP_PER_CORE * P], F32)

            for e in range(EXP_PER_CORE):
                wt = wpool.tile([128, WCOLS], F32)
                nc.sync.dma_start(wt[:], w_d[e])
                w1t_a = wt[:, 0:128]  # (W1^T)[:, :128]   lhsT for h[:128]
                w1t_b = wt[:, 128:256]  # (W1^T)[:, 128:]   lhsT for h[128:]
                w2t_a = wt[:, 256:384]  # (W2^T)[:128, :]   K = h[:128]
                w2t_b = wt[:, 384:512]  # (W2^T)[128:, :]   K = h[128:]
                b1a = wt[:, 512:513]
                b1b = wt[:, 513:514]
                b2 = wt[:, 514:515]

                if MM_F32R:
                    w1t_a = w1t_a.bitcast(F32R)
                    w1t_b = w1t_b.bitcast(F32R)
                    w2t_a = w2t_a.bitcast(F32R)
                    w2t_b = w2t_b.bitcast(F32R)

                for p0 in range(0, P, 512):
                    pc = min(512, P - p0)
                    xe = xt[:, e * P + p0 : e * P + p0 + pc]
                    if MM_F32R:
                        xe = xe.bitcast(F32R)
                    ph_a = psum.tile([128, pc], F32, tag="ph_a")
                    nc.tensor.matmul(ph_a[:], w1t_a, xe, start=True, stop=True)
                    ph_b = psum.tile([128, pc], F32, tag="ph_b")
                    nc.tensor.matmul(ph_b[:], w1t_b, xe, start=True, stop=True)

                    # bias + relu, split across ACT and DVE to balance engines
                    ha = hpool.tile([128, pc], F32, tag="ha")
                    nc.scalar.activation(ha[:], ph_a[:], relu, bias=b1a)
                    hb = hpool.tile([128, pc], F32, tag="hb")
                    nc.vector.tensor_scalar(hb[:], ph_b[:], b1b, 0.0, add, amax)

                    py = psum.tile([128, pc], F32, tag="py")
                    ha_mm = ha[:].bitcast(F32R) if MM_F32R else ha[:]
                    hb_mm = hb[:].bitcast(F32R) if MM_F32R else hb[:]
                    nc.tensor.matmul(py[:], w2t_a, ha_mm, start=True, stop=False)
                    nc.tensor.matmul(py[:], w2t_b, hb_mm, start=False, stop=True)

                    yo = yt[:, e * P + p0 : e * P + p0 + pc]
                    if e % 2 == 0:
                        nc.scalar.activation(yo, py[:], ident, bias=b2)
                    else:
                        nc.vector.tensor_scalar(yo, py[:], b2, None, add)

            nc.sync.dma_start(yt_d[:], yt[:])
    return nc


def _pack_weights(quant_fn: np.ndarray, first_idx: dict[int, int]) -> np.ndarray:
    wpack = np.zeros((NUM_EMB, 128, WCOLS), dtype=np.float32)
    for e, row in first_idx.items():
        q = quant_fn[row]
        w1 = q[:_I0].reshape(H_SIZE, X_SIZE)  # [256, 128]
        b1 = q[_I0:_I1]  # [256]
        w2 = q[_I1:_I2].reshape(OUT_SIZE, H_SIZE)  # [128, 256]
        b2 = q[_I2:]  # [128]
        wpack[e, :, 0:256] = w1.T
        w2t = np.ascontiguousarray(w2.T)  # [256, 128]
        wpack[e, :, 256:384] = w2t[:128]
        wpack[e, :, 384:512] = w2t[128:]
        wpack[e, :, 512] = b1[:128]
        wpack[e, :, 513] = b1[128:]
        wpack[e, :, 514] = b2
    return wpack


def _run(quant_fn, x, emb_idx, trace=False, tmpdir=None):
    quant_fn = np.asarray(quant_fn, dtype=np.float32)
    x = np.asarray(x, dtype=np.float32)
    idx = np.asarray(emb_idx).astype(np.int64).ravel()
    n = x.shape[0]

    counts = np.bincount(idx, minlength=NUM_EMB)
    order = np.argsort(idx, kind="stable")
    starts = np.zeros(NUM_EMB + 1, dtype=np.int64)
    np.cumsum(counts, out=starts[1:])
    P = int(counts.max())
    P = max((P + 7) // 8 * 8, 8)

    uniq, first = np.unique(idx, return_index=True)
    wpack = _pack_weights(quant_fn, {int(e): int(r) for e, r in zip(uniq, first)})

    xs = x[order]  # [n, 128] grouped by expert
    xt_all = np.zeros((NUM_EMB, 128, P), dtype=np.float32)
    for e in range(NUM_EMB):
        c = counts[e]
        if c:
            xt_all[e, :, :c] = xs[starts[e] : starts[e + 1]].T

    in_maps = []
    for k in range(N_CORES):
        sl = slice(k * EXP_PER_CORE, (k + 1) * EXP_PER_CORE)
        xt_core = np.ascontiguousarray(
            xt_all[sl].transpose(1, 0, 2).reshape(128, EXP_PER_CORE * P)
        )
        in_maps.append({"w": np.ascontiguousarray(wpack[sl]), "xt": xt_core})

    key = (P, RAW_BASS)
    nc = _PROGRAM_CACHE.get(key)
    if nc is None:
        nc = _build_program_raw(P) if RAW_BASS else _build_program(P)
        _PROGRAM_CACHE[key] = nc

    res = run_bass_kernel_spmd(
        nc, in_maps, list(range(N_CORES)), trace=trace, tmpdir=tmpdir
    )

    out = np.empty((n, OUT_SIZE), dtype=np.float32)
    for k in range(N_CORES):
        yt = res.results[k]["yt"]
        for j in range(EXP_PER_CORE):
            e = k * EXP_PER_CORE + j
            c = counts[e]
            if c:
                out[order[starts[e] : starts[e + 1]]] = yt[:, j * P : j * P + c].T
    return out, res


def kernel(**inputs) -> np.ndarray:
    out, _ = _run(inputs["quant_fn"], inputs["x"], inputs["emb_idx"])
    return out


# revision 42
# speedup vs baseline: 1.0143x; 1.0143x over previous
"""MoE-routed batched decoder kernel for Trainium2 (8 NeuronCores, SPMD).

Problem: per-sample 2-layer MLP (128 -> 256 relu -> 128) where each sample's
flat weight vector quant_fn[n] is one of 64 codebook rows selected by
emb_idx[n] (VQ semantics: rows are identical within an expert group).

Strategy (expert-parallel MoE routing):
  host: route tokens by emb_idx (stable sort), dedupe weights to the 64
        unique codebook rows (16.9 MB instead of 1.08 GB), pre-transpose
        into PE lhsT layout, pad each expert's token group to P columns;
  device (SPMD x8): core k owns experts [8k, 8k+8); per expert two
        fp32 matmuls for layer 1 (W1^T halves), fused bias+relu, two
        accumulating matmuls for layer 2 (W2^T halves), fused bias add;
  host: inverse-permute per-expert outputs back to token order.
"""

import sys
import types

import numpy as np

for _p in ("/opt/trn_rl_repo", "/root/.axon_site/_ro/trn_rl_repo"):
    if _p not in sys.path:
        sys.path.append(_p)

import concourse.bass as bass
import concourse.tile as tile
from concourse import mybir
from concourse.bass_utils import run_bass_kernel_spmd
from bass_rust import ScopedClock

X_SIZE = 128
H_SIZE = 256
OUT_SIZE = 128
NUM_EMB = 64
N_CORES = 8
EXP_PER_CORE = NUM_EMB // N_CORES
# packed per-expert weight block columns: W1^T [128,256] | W2^T upper [128,128]
# | W2^T lower [128,128] | b1[:128] | b1[128:] | b2 | pad
WCOLS = 516
_I0 = H_SIZE * X_SIZE
_I1 = _I0 + H_SIZE
_I2 = _I1 + OUT_SIZE * H_SIZE

F32 = mybir.dt.float32
F32R = mybir.dt.float32r
MM_F32R = False  # float32r needs explicit rounding ops and loses precision; keep fp32


def _install_axon_shims():
    """Make trace=True survivable in this container: register the ctypes NTFF
    hook for the missing antenv.axon_hooks module, and keep profile artifacts
    local (no bucket creds in the sandbox)."""
    try:
        import antenv

        if "antenv.axon_hooks" not in sys.modules:
            shim = types.ModuleType("antenv.axon_hooks")
            holder = {}
            shim.set_axon_ntff_profile_hook = lambda h: holder.__setitem__("h", h)
            shim.get_axon_ntff_profile_hook = lambda: holder.get("h")
            sys.modules["antenv.axon_hooks"] = shim
            antenv.axon_hooks = shim
            import trn_agent_boot.trn_boot as _tb

            shim.set_axon_ntff_profile_hook(
                _tb._ntff_profile_via_ctypes("/opt/axon/libaxon_pjrt.so")
            )
        import concourse.bass_utils as _bu

        _bu.upload_artifacts = lambda tmpdir: "local://" + tmpdir
    except Exception:
        pass


def _patched_drain_and_barrier(self, tick_clock, wait_clock):
    # The stock implementation piles every outstanding sem wait onto the single
    # tail Drain CTRL, which overflows this walrus build's sync-wait slots.
    # Emit one wait_ge per semaphore on the sync engine, then a bare drain.
    probe = mybir.InstNoOp(
        name="tile_drain_probe", engine=mybir.EngineType.SP, bass_nofuse=True
    )
    wait_clock.add_sem_waits(probe, ScopedClock({None: tick_clock.global_clock}))
    by_name = {h.name: h for h in self.sems.allocated().values()}
    for w in probe.sync_info.on_wait if probe.sync_info else []:
        self.nc.sync.wait_ge(by_name[w.ant_name], w.wait_value)
    self.nc.sync.drain()
    self.nc.all_engine_barrier()
    popped = self.nc._tile_sem_poison_stack.pop()
    assert popped is self._sem_poison
    self.nc.clear_and_free_semaphores(list(self.sems.allocated().values()))
    self.nc.all_engine_barrier()


_orig_lower_ordered_insts = tile.TileContext._lower_ordered_insts
_waitsplit_counter = [0]


def _lower_with_wait_split(self, ordered):
    # This walrus build rejects instructions carrying more than one sync wait
    # ("Too many sync wait commands"). Hoist excess waits into dedicated
    # single-wait NoOps on the same engine, immediately before the offender.
    for bb_name, insts in ordered.items():
        new = []
        for inst in insts:
            si = inst.sync_info
            if si is not None and len(si.on_wait) > 1:
                waits = list(si.on_wait)
                for w in waits[:-1]:
                    _waitsplit_counter[0] += 1
                    new.append(
                        mybir.InstNoOp(
                            name=f"I-waitsplit-{_waitsplit_counter[0]}",
                            engine=inst.engine,
                            sync_info=mybir.SyncInfo(on_wait=[w], on_update=[]),
                            bass_nofuse=True,
                        )
                    )
                inst.sync_info = mybir.SyncInfo(
                    on_wait=[waits[-1]], on_update=list(si.on_update)
                )
            new.append(inst)
        ordered[bb_name] = new
    return _orig_lower_ordered_insts(self, ordered)


tile.TileContext._lower_ordered_insts = _lower_with_wait_split
tile.TileContext._drain_and_barrier = _patched_drain_and_barrier
_install_axon_shims()

_PROGRAM_CACHE: dict[tuple, bass.Bass] = {}
RAW_BASS = True  # hand-scheduled pipeline (no TileContext) — much smaller head/tail
LEAN_CTOR = True  # strip Bass-ctor preamble/barrier (engine-register init unused here)


class _LeanBass(bass.Bass):
    """Bass that skips the ctor-emitted engine preambles and the post-const
    all-engine barrier. Our kernel does its own cross-engine sync from scratch
    semaphores, and the preamble register inits aren't used by the instruction
    mix here (DMA / matmul / activation / tensor_scalar)."""

    def all_engine_barrier(self, **kw):
        if getattr(self, "_lean_done", False):
            return super().all_engine_barrier(**kw)


def _make_bass() -> bass.Bass:
    if not LEAN_CTOR:
        return bass.Bass(
            "TRN2", target_bir_lowering=False, debug=False, num_devices=N_CORES
        )
    orig_preamble = bass.BassEngine.preamble
    bass.BassEngine.preamble = lambda self: None
    try:
        nc = _LeanBass(
            "TRN2", target_bir_lowering=False, debug=False, num_devices=N_CORES
        )
    finally:
        bass.BassEngine.preamble = orig_preamble
    nc._lean_done = True
    return nc


def _build_program_raw(P: int) -> bass.Bass:
    """Static 4-engine pipeline with manual semaphores.

    sync   : x DMA, per-expert weight DMA in, per-expert y DMA out
    tensor : per expert, 4 fp32 matmuls (2x L1 halves, 2x accumulating L2)
    scalar : relu+bias for the first H half; y bias+copy for even experts
    vector : relu+bias for the second H half; y bias+copy for odd experts
    PSUM   : ph_a/ph_b/py double-buffered by expert parity (6 banks)
    """
    E = EXP_PER_CORE
    nc = _make_bass()
    w_d = nc.declare_dram_parameter("w", [E, 128, WCOLS], F32, isOutput=False)
    xt_d = nc.declare_dram_parameter("xt", [128, E * P], F32, isOutput=False)
    yt_d = nc.declare_dram_parameter("yt", [128, E * P], F32, isOutput=True)

    relu = mybir.ActivationFunctionType.Relu
    ident = mybir.ActivationFunctionType.Identity
    add = mybir.AluOpType.add
    amax = mybir.AluOpType.max

    xt = nc.alloc_sbuf_tensor("xt_sb", [128, E * P], F32).ap()
    yt = nc.alloc_sbuf_tensor("yt_sb", [128, E * P], F32).ap()
    w_all = nc.alloc_sbuf_tensor("w_sb", [128, E * WCOLS], F32).ap()
    w_sb = [w_all[:, e * WCOLS : (e + 1) * WCOLS] for e in range(E)]
    scr = nc.alloc_sbuf_tensor("scratch", [128, 512], F32).ap()
    ha = [nc.alloc_sbuf_tensor(f"ha{i}", [128, P], F32).ap() for i in range(2)]
    hb = [nc.alloc_sbuf_tensor(f"hb{i}", [128, P], F32).ap() for i in range(2)]
    pha = [nc.alloc_psum_tensor(f"pha{i}", [128, 512], F32).ap() for i in range(3)]
    phb = [nc.alloc_psum_tensor(f"phb{i}", [128, 512], F32).ap() for i in range(3)]
    ppy = [nc.alloc_psum_tensor(f"ppy{i}", [128, 512], F32).ap() for i in range(2)]
    # DRAM weights viewed [partition, expert, col] for paired-expert DMAs
    w_dp = w_d.ap().rearrange("e p c -> p e c")

    from contextlib import ExitStack

    with ExitStack() as st:
        sem = lambda name: st.enter_context(nc.semaphore(name))
        s_x = sem("s_x")
        s_xr = sem("s_xr")
        s_w = [sem(f"s_w{e}") for e in range(E)]
        s_ph = sem("s_ph")
        s_ha, s_hb = sem("s_ha"), sem("s_hb")
        s_py = sem("s_py")
        s_ye, s_yo = sem("s_ye"), sem("s_yo")
        s_out = sem("s_out")
        block = st.enter_context(nc.Block())

        @block.sync
        def _(sync):
            # per-expert (x slice, weights) bundles in expert order: each expert
            # becomes runnable ~0.77us after the previous one
            sync.dma_start(out=xt[:, : 2 * P], in_=xt_d[:, : 2 * P]).then_inc(s_x, 16)
            sync.dma_start(out=w_sb[0][:], in_=w_dp[:, 0, :]).then_inc(s_w[0], 16)
            sync.dma_start(out=w_sb[1][:], in_=w_dp[:, 1, :]).then_inc(s_w[1], 16)
            for e in range(2, E):
                sync.dma_start(
                    out=xt[:, e * P : (e + 1) * P], in_=xt_d[:, e * P : (e + 1) * P]
                ).then_inc(s_w[e], 16)
                sync.dma_start(
                    out=w_sb[e][:], in_=w_dp[:, e, :]
                ).then_inc(s_w[e], 16)
            for j in range(E // 2):
                # two experts' outputs per DMA (even parity fused, odd standalone)
                sync.wait_ge(s_yo, j + 1)
                dma = sync.dma_start(
                    out=yt_d[:, 2 * j * P : (2 * j + 2) * P],
                    in_=yt[:, 2 * j * P : (2 * j + 2) * P],
                )
                dma._wait_ge(s_ye, j + 1)
                dma.then_inc(s_out, 16)
            sync.wait_ge(s_out, 16 * (E // 2))

        @block.tensor
        def _(tensor):
            # Software-pipelined: L1 of expert e+1 issues before L2 of expert e,
            # so the relu handoff (PE -> ACT/DVE -> PE) hides behind L1 matmuls.
            # Each logical matmul is col-split into two concurrent M=64 tiles
            # (tile_position (0,0)/(0,64)): the 64-col LDWEIGHTS pair overlaps,
            # halving the weight-load bound that dominates fp32 passes.
            def mm_split(out, lhsT, rhs, start, stop, wait=None, inc=None):
                m1 = nc.tensor.matmul(
                    out[0:64, :P], lhsT[:, 0:64], rhs,
                    start=start, stop=stop, tile_position=(0, 0),
                )
                if wait is not None:
                    m1._wait_ge(*wait)
                m2 = nc.tensor.matmul(
                    out[64:128, :P], lhsT[:, 64:128], rhs,
                    start=start, stop=stop, tile_position=(0, 64),
                )
                if inc is not None:
                    m2.then_inc(*inc)

            def l1(e):
                i = e % 3
                w = w_sb[e]
                xe = xt[:, e * P : (e + 1) * P]
                if e >= 3:
                    # ph bank of e-3 free once its relus have read it
                    tensor.wait_ge(s_ha, e - 2)
                mm_split(
                    pha[i], w[:, 0:128], xe, True, True,
                    wait=(s_w[e], 32 if e >= 2 else 16),
                )
                mm_split(
                    phb[i], w[:, 128:256], xe, True, True,
                    wait=(s_hb, e - 2) if e >= 3 else None, inc=(s_ph, 1),
                )

            def l2(e):
                i = e % 2
                w = w_sb[e]
                if e >= 2:
                    # py bank of e-2 free once its y copy completed
                    tensor.wait_ge(s_ye if e % 2 == 0 else s_yo, e // 2)
                mm_split(
                    ppy[i], w[:, 256:384], ha[i][:], True, False, wait=(s_ha, e + 1)
                )
                mm_split(
                    ppy[i], w[:, 384:512], hb[i][:], False, True,
                    wait=(s_hb, e + 1), inc=(s_py, 1),
                )

            # dummy matmuls on scratch data ramp the HAM clock gate to 2.4GHz
            # while the first input DMAs land (~3us of sustained PE activity);
            # they write ppy[0], which l2(0) later resets with start=True
            for _ in range(7):
                nc.tensor.matmul(
                    ppy[0][:, :96], scr[:, :128], scr[:, :96], start=True, stop=True
                )
            tensor.wait_ge(s_x, 16)
            l1(0)
            l1(1)
            l1(2)
            for e in range(E):
                if e + 3 <= E - 1:
                    l1(e + 3)
                l2(e)

        # y-copy ops are emitted two experts late so the relu chain (which gates
        # the PE) never queues behind a y op on the same in-order engine.
        def y_scalar(e):
            y = nc.scalar.activation(
                yt[:, e * P : (e + 1) * P],
                ppy[e % 2][:, :P],
                ident,
                bias=w_sb[e][:, 514:515],
            )
            y._wait_ge(s_py, e + 1)
            y.then_inc(s_ye)

        def y_vector(e):
            y = nc.vector.tensor_scalar(
                yt[:, e * P : (e + 1) * P],
                ppy[e % 2][:, :P],
                w_sb[e][:, 514:515],
                None,
                add,
            )
            y._wait_ge(s_py, e + 1)
            y.then_inc(s_yo)

        @block.scalar
        def _(scalar):
            for e in range(E):
                i = e % 2
                if e >= 2:
                    scalar.wait_ge(s_py, e - 1)  # ha buf consumed by e-2's L2
                act = nc.scalar.activation(
                    ha[i][:], pha[e % 3][:, :P], relu, bias=w_sb[e][:, 512:513]
                )
                act._wait_ge(s_ph, e + 1)
                act.then_inc(s_ha)
                if e >= 2 and (e - 2) % 2 == 0:
                    y_scalar(e - 2)
            y_scalar(E - 2)

        @block.vector
        def _(vector):
            for e in range(E):
                i = e % 2
                if e >= 2:
                    vector.wait_ge(s_py, e - 1)
                ts = nc.vector.tensor_scalar(
                    hb[i][:], phb[e % 3][:, :P], w_sb[e][:, 513:514], 0.0, add, amax
                )
                ts._wait_ge(s_ph, e + 1)
                ts.then_inc(s_hb)
                if e >= 2 and (e - 2) % 2 == 1:
                    y_vector(e - 2)
            y_vector(E - 1)

    return nc


def _build_program(P: int) -> bass.Bass:
    nc = bass.Bass("TRN2", target_bir_lowering=False, debug=False, num_devices=N_CORES)
    w_d = nc.declare_dram_parameter(
        "w", [EXP_PER_CORE, 128, WCOLS], F32, isOutput=False
    )
    xt_d = nc.declare_dram_parameter("xt", [128, EXP_PER_CORE * P], F32, isOutput=False)
    yt_d = nc.declare_dram_parameter("yt", [128, EXP_PER_CORE * P], F32, isOutput=True)

    relu = mybir.ActivationFunctionType.Relu
    ident = mybir.ActivationFunctionType.Identity
    add = mybir.AluOpType.add
    amax = mybir.AluOpType.max

    with tile.TileContext(nc) as tc:
        with (
            tc.tile_pool(name="wpool", bufs=8) as wpool,
            tc.tile_pool(name="xpool", bufs=1) as xpool,
            tc.tile_pool(name="hpool", bufs=6) as hpool,
            tc.tile_pool(name="ypool", bufs=1) as ypool,
            tc.tile_pool(name="psum", bufs=2, space="PSUM") as psum,
        ):
            xt = xpool.tile([128, EXP_PER_CORE * P], F32)
            nc.sync.dma_start(xt[:], xt_d[:])
            yt = ypool.tile([128, EXP_PER_CORE * P], F32)

            for e in range(EXP_PER_CORE):
                wt = wpool.tile([128, WCOLS], F32)
                nc.sync.dma_start(wt[:], w_d[e])
                w1t_a = wt[:, 0:128]  # (W1^T)[:, :128]   lhsT for h[:128]
                w1t_b = wt[:, 128:256]  # (W1^T)[:, 128:]   lhsT for h[128:]
                w2t_a = wt[:, 256:384]  # (W2^T)[:128, :]   K = h[:128]
                w2t_b = wt[:, 384:512]  # (W2^T)[128:, :]   K = h[128:]
                b1a = wt[:, 512:513]
                b1b = wt[:, 513:514]
                b2 = wt[:, 514:515]

                if MM_F32R:
                    w1t_a = w1t_a.bitcast(F32R)
                    w1t_b = w1t_b.bitcast(F32R)
                    w2t_a = w2t_a.bitcast(F32R)
                    w2t_b = w2t_b.bitcast(F32R)

                for p0 in range(0, P, 512):
                    pc = min(512, P - p0)
                    xe = xt[:, e * P + p0 : e * P + p0 + pc]
                    if MM_F32R:
                        xe = xe.bitcast(F32R)
                    ph_a = psum.tile([128, pc], F32, tag="ph_a")
                    nc.tensor.matmul(ph_a[:], w1t_a, xe, start=True, stop=True)
                    ph_b = psum.tile([128, pc], F32, tag="ph_b")
                    nc.tensor.matmul(ph_b[:], w1t_b, xe, start=True, stop=True)

                    # bias + relu, split across ACT and DVE to balance engines
                    ha = hpool.tile([128, pc], F32, tag="ha")
                    nc.scalar.activation(ha[:], ph_a[:], relu, bias=b1a)
                    hb = hpool.tile([128, pc], F32, tag="hb")
                    nc.vector.tensor_scalar(hb[:], ph_b[:], b1b, 0.0, add, amax)

                    py = psum.tile([128, pc], F32, tag="py")
                    ha_mm = ha[:].bitcast(F32R) if MM_F32R else ha[:]
                    hb_mm = hb[:].bitcast(F32R) if MM_F32R else hb[:]
                    nc.tensor.matmul(py[:], w2t_a, ha_mm, start=True, stop=False)
                    nc.tensor.matmul(py[:], w2t_b, hb_mm, start=False, stop=True)

                    yo = yt[:, e * P + p0 : e * P + p0 + pc]
                    if e % 2 == 0:
                        nc.scalar.activation(yo, py[:], ident, bias=b2)
                    else:
                        nc.vector.tensor_scalar(yo, py[:], b2, None, add)

            nc.sync.dma_start(yt_d[:], yt[:])
    return nc


def _pack_weights(quant_fn: np.ndarray, first_idx: dict[int, int]) -> np.ndarray:
    wpack = np.zeros((NUM_EMB, 128, WCOLS), dtype=np.float32)
    for e, row in first_idx.items():
        q = quant_fn[row]
        w1 = q[:_I0].reshape(H_SIZE, X_SIZE)  # [256, 128]
        b1 = q[_I0:_I1]  # [256]
        w2 = q[_I1:_I2].reshape(OUT_SIZE, H_SIZE)  # [128, 256]
        b2 = q[_I2:]  # [128]
        wpack[e, :, 0:256] = w1.T
        w2t = np.ascontiguousarray(w2.T)  # [256, 128]
        wpack[e, :, 256:384] = w2t[:128]
        wpack[e, :, 384:512] = w2t[128:]
        wpack[e, :, 512] = b1[:128]
        wpack[e, :, 513] = b1[128:]
        wpack[e, :, 514] = b2
    return wpack


def _run(quant_fn, x, emb_idx, trace=False, tmpdir=None):
    quant_fn = np.asarray(quant_fn, dtype=np.float32)
    x = np.asarray(x, dtype=np.float32)
    idx = np.asarray(emb_idx).astype(np.int64).ravel()
    n = x.shape[0]

    counts = np.bincount(idx, minlength=NUM_EMB)
    order = np.argsort(idx, kind="stable")
    starts = np.zeros(NUM_EMB + 1, dtype=np.int64)
    np.cumsum(counts, out=starts[1:])
    P = int(counts.max())
    P = max((P + 7) // 8 * 8, 8)

    uniq, first = np.unique(idx, return_index=True)
    wpack = _pack_weights(quant_fn, {int(e): int(r) for e, r in zip(uniq, first)})

    xs = x[order]  # [n, 128] grouped by expert
    xt_all = np.zeros((NUM_EMB, 128, P), dtype=np.float32)
    for e in range(NUM_EMB):
        c = counts[e]
        if c:
            xt_all[e, :, :c] = xs[starts[e] : starts[e + 1]].T

    in_maps = []
    for k in range(N_CORES):
        sl = slice(k * EXP_PER_CORE, (k + 1) * EXP_PER_CORE)
        xt_core = np.ascontiguousarray(
            xt_all[sl].transpose(1, 0, 2).reshape(128, EXP_PER_CORE * P)
        )
        in_maps.append({"w": np.ascontiguousarray(wpack[sl]), "xt": xt_core})

    key = (P, RAW_BASS)
    nc = _PROGRAM_CACHE.get(key)
    if nc is None:
        nc = _build_program_raw(P) if RAW_BASS else _build_program(P)
        _PROGRAM_CACHE[key] = nc

    res = run_bass_kernel_spmd(
        nc, in_maps, list(range(N_CORES)), trace=trace, tmpdir=tmpdir
    )

    out = np.empty((n, OUT_SIZE), dtype=np.float32)
    for k in range(N_CORES):
        yt = res.results[k]["yt"]
        for j in range(EXP_PER_CORE):
            e = k * EXP_PER_CORE + j
            c = counts[e]
            if c:
                out[order[starts[e] : starts[e + 1]]] = yt[:, j * P : j * P + c].T
    return out, res


def kernel(**inputs) -> np.ndarray:
    out, _ = _run(inputs["quant_fn"], inputs["x"], inputs["emb_idx"])
    return out


# revision 45
# speedup vs baseline: 1.0771x; 1.0619x over previous
"""MoE-routed batched decoder kernel for Trainium2 (8 NeuronCores, SPMD).

Problem: per-sample 2-layer MLP (128 -> 256 relu -> 128) where each sample's
flat weight vector quant_fn[n] is one of 64 codebook rows selected by
emb_idx[n] (VQ semantics: rows are identical within an expert group).

Strategy (expert-parallel MoE routing):
  host: route tokens by emb_idx (stable sort), dedupe weights to the 64
        unique codebook rows (16.9 MB instead of 1.08 GB), pre-transpose
        into PE lhsT layout, pad each expert's token group to P columns;
  device (SPMD x8): core k owns experts [8k, 8k+8); per expert two
        fp32 matmuls for layer 1 (W1^T halves), fused bias+relu, two
        accumulating matmuls for layer 2 (W2^T halves), fused bias add;
  host: inverse-permute per-expert outputs back to token order.
"""

import sys
import types

import numpy as np

for _p in ("/opt/trn_rl_repo", "/root/.axon_site/_ro/trn_rl_repo"):
    if _p not in sys.path:
        sys.path.append(_p)

import concourse.bass as bass
import concourse.tile as tile
from concourse import mybir
from concourse.bass_utils import run_bass_kernel_spmd
from bass_rust import ScopedClock

X_SIZE = 128
H_SIZE = 256
OUT_SIZE = 128
NUM_EMB = 64
N_CORES = 8
EXP_PER_CORE = NUM_EMB // N_CORES
# packed per-expert weight block columns: W1^T [128,256] | W2^T upper [128,128]
# | W2^T lower [128,128] | b1[:128] | b1[128:] | b2 | pad
WCOLS = 516
_I0 = H_SIZE * X_SIZE
_I1 = _I0 + H_SIZE
_I2 = _I1 + OUT_SIZE * H_SIZE

F32 = mybir.dt.float32
F32R = mybir.dt.float32r
MM_F32R = False  # float32r needs explicit rounding ops and loses precision; keep fp32


def _install_axon_shims():
    """Make trace=True survivable in this container: register the ctypes NTFF
    hook for the missing antenv.axon_hooks module, and keep profile artifacts
    local (no bucket creds in the sandbox)."""
    try:
        import antenv

        if "antenv.axon_hooks" not in sys.modules:
            shim = types.ModuleType("antenv.axon_hooks")
            holder = {}
            shim.set_axon_ntff_profile_hook = lambda h: holder.__setitem__("h", h)
            shim.get_axon_ntff_profile_hook = lambda: holder.get("h")
            sys.modules["antenv.axon_hooks"] = shim
            antenv.axon_hooks = shim
            import trn_agent_boot.trn_boot as _tb

            shim.set_axon_ntff_profile_hook(
                _tb._ntff_profile_via_ctypes("/opt/axon/libaxon_pjrt.so")
            )
        import concourse.bass_utils as _bu

        _bu.upload_artifacts = lambda tmpdir: "local://" + tmpdir
    except Exception:
        pass


def _patched_drain_and_barrier(self, tick_clock, wait_clock):
    # The stock implementation piles every outstanding sem wait onto the single
    # tail Drain CTRL, which overflows this walrus build's sync-wait slots.
    # Emit one wait_ge per semaphore on the sync engine, then a bare drain.
    probe = mybir.InstNoOp(
        name="tile_drain_probe", engine=mybir.EngineType.SP, bass_nofuse=True
    )
    wait_clock.add_sem_waits(probe, ScopedClock({None: tick_clock.global_clock}))
    by_name = {h.name: h for h in self.sems.allocated().values()}
    for w in probe.sync_info.on_wait if probe.sync_info else []:
        self.nc.sync.wait_ge(by_name[w.ant_name], w.wait_value)
    self.nc.sync.drain()
    self.nc.all_engine_barrier()
    popped = self.nc._tile_sem_poison_stack.pop()
    assert popped is self._sem_poison
    self.nc.clear_and_free_semaphores(list(self.sems.allocated().values()))
    self.nc.all_engine_barrier()


_orig_lower_ordered_insts = tile.TileContext._lower_ordered_insts
_waitsplit_counter = [0]


def _lower_with_wait_split(self, ordered):
    # This walrus build rejects instructions carrying more than one sync wait
    # ("Too many sync wait commands"). Hoist excess waits into dedicated
    # single-wait NoOps on the same engine, immediately before the offender.
    for bb_name, insts in ordered.items():
        new = []
        for inst in insts:
            si = inst.sync_info
            if si is not None and len(si.on_wait) > 1:
                waits = list(si.on_wait)
                for w in waits[:-1]:
                    _waitsplit_counter[0] += 1
                    new.append(
                        mybir.InstNoOp(
                            name=f"I-waitsplit-{_waitsplit_counter[0]}",
                            engine=inst.engine,
                            sync_info=mybir.SyncInfo(on_wait=[w], on_update=[]),
                            bass_nofuse=True,
                        )
                    )
                inst.sync_info = mybir.SyncInfo(
                    on_wait=[waits[-1]], on_update=list(si.on_update)
                )
            new.append(inst)
        ordered[bb_name] = new
    return _orig_lower_ordered_insts(self, ordered)


tile.TileContext._lower_ordered_insts = _lower_with_wait_split
tile.TileContext._drain_and_barrier = _patched_drain_and_barrier
_install_axon_shims()

_PROGRAM_CACHE: dict[tuple, bass.Bass] = {}
RAW_BASS = True  # hand-scheduled pipeline (no TileContext) — much smaller head/tail
LEAN_CTOR = True  # strip Bass-ctor preamble/barrier (engine-register init unused here)


class _LeanBass(bass.Bass):
    """Bass that skips the ctor-emitted engine preambles and the post-const
    all-engine barrier. Our kernel does its own cross-engine sync from scratch
    semaphores, and the preamble register inits aren't used by the instruction
    mix here (DMA / matmul / activation / tensor_scalar)."""

    def all_engine_barrier(self, **kw):
        if getattr(self, "_lean_done", False):
            return super().all_engine_barrier(**kw)


def _make_bass() -> bass.Bass:
    if not LEAN_CTOR:
        return bass.Bass(
            "TRN2", target_bir_lowering=False, debug=False, num_devices=N_CORES
        )
    orig_preamble = bass.BassEngine.preamble
    bass.BassEngine.preamble = lambda self: None
    try:
        nc = _LeanBass(
            "TRN2", target_bir_lowering=False, debug=False, num_devices=N_CORES
        )
    finally:
        bass.BassEngine.preamble = orig_preamble
    nc._lean_done = True
    return nc


def _build_program_raw(P: int) -> bass.Bass:
    """Static 4-engine pipeline with manual semaphores.

    sync   : x DMA, per-expert weight DMA in, per-expert y DMA out
    tensor : per expert, 4 fp32 matmuls (2x L1 halves, 2x accumulating L2)
    scalar : relu+bias for the first H half; y bias+copy for even experts
    vector : relu+bias for the second H half; y bias+copy for odd experts
    PSUM   : ph_a/ph_b/py double-buffered by expert parity (6 banks)
    """
    E = EXP_PER_CORE
    nc = _make_bass()
    w_d = nc.declare_dram_parameter("w", [E, 128, WCOLS], F32, isOutput=False)
    xt_d = nc.declare_dram_parameter("xt", [128, E * P], F32, isOutput=False)
    yt_d = nc.declare_dram_parameter("yt", [128, E * P], F32, isOutput=True)

    relu = mybir.ActivationFunctionType.Relu
    ident = mybir.ActivationFunctionType.Identity
    add = mybir.AluOpType.add
    amax = mybir.AluOpType.max

    xt = nc.alloc_sbuf_tensor("xt_sb", [128, E * P], F32).ap()
    yt = nc.alloc_sbuf_tensor("yt_sb", [128, E * P], F32).ap()
    w_all = nc.alloc_sbuf_tensor("w_sb", [128, E * WCOLS], F32).ap()
    w_sb = [w_all[:, e * WCOLS : (e + 1) * WCOLS] for e in range(E)]
    scr = nc.alloc_sbuf_tensor("scratch", [128, 512], F32).ap()
    ha = [nc.alloc_sbuf_tensor(f"ha{i}", [128, P], F32).ap() for i in range(2)]
    hb = [nc.alloc_sbuf_tensor(f"hb{i}", [128, P], F32).ap() for i in range(2)]
    pha = [nc.alloc_psum_tensor(f"pha{i}", [128, 512], F32).ap() for i in range(3)]
    phb = [nc.alloc_psum_tensor(f"phb{i}", [128, 512], F32).ap() for i in range(3)]
    ppy = [nc.alloc_psum_tensor(f"ppy{i}", [128, 512], F32).ap() for i in range(2)]
    # DRAM weights viewed [partition, expert, col] for paired-expert DMAs
    w_dp = w_d.ap().rearrange("e p c -> p e c")

    from contextlib import ExitStack

    with ExitStack() as st:
        sem = lambda name: st.enter_context(nc.semaphore(name))
        s_x = sem("s_x")
        s_xr = sem("s_xr")
        s_w = [sem(f"s_w{e}") for e in range(E)]
        s_ph = sem("s_ph")
        s_ha, s_hb = sem("s_ha"), sem("s_hb")
        s_py = sem("s_py")
        s_ye, s_yo = sem("s_ye"), sem("s_yo")
        s_out = sem("s_out")
        block = st.enter_context(nc.Block())

        @block.sync
        def _(sync):
            # weights in expert order on the sync HWDGE queue; x remainder and
            # output DMAs ride the otherwise-idle gpsimd queue in parallel
            sync.dma_start(out=xt[:, : 2 * P], in_=xt_d[:, : 2 * P]).then_inc(s_x, 16)
            for e in range(E):
                sync.dma_start(
                    out=w_sb[e][:], in_=w_dp[:, e, :]
                ).then_inc(s_w[e], 16)

        @block.gpsimd
        def _(gpsimd):
            gpsimd.dma_start(out=xt[:, 2 * P :], in_=xt_d[:, 2 * P :]).then_inc(
                s_xr, 16
            )
            for j in range(E // 2):
                # two experts' outputs per DMA (even parity fused, odd standalone)
                gpsimd.wait_ge(s_yo, j + 1)
                dma = gpsimd.dma_start(
                    out=yt_d[:, 2 * j * P : (2 * j + 2) * P],
                    in_=yt[:, 2 * j * P : (2 * j + 2) * P],
                )
                dma._wait_ge(s_ye, j + 1)
                dma.then_inc(s_out, 16)
            gpsimd.wait_ge(s_out, 16 * (E // 2))

        @block.tensor
        def _(tensor):
            # Software-pipelined: L1 of expert e+1 issues before L2 of expert e,
            # so the relu handoff (PE -> ACT/DVE -> PE) hides behind L1 matmuls.
            # Each logical matmul is col-split into two concurrent M=64 tiles
            # (tile_position (0,0)/(0,64)): the 64-col LDWEIGHTS pair overlaps,
            # halving the weight-load bound that dominates fp32 passes.
            def mm_split(out, lhsT, rhs, start, stop, wait=None, inc=None):
                m1 = nc.tensor.matmul(
                    out[0:64, :P], lhsT[:, 0:64], rhs,
                    start=start, stop=stop, tile_position=(0, 0),
                )
                if wait is not None:
                    m1._wait_ge(*wait)
                m2 = nc.tensor.matmul(
                    out[64:128, :P], lhsT[:, 64:128], rhs,
                    start=start, stop=stop, tile_position=(0, 64),
                )
                if inc is not None:
                    m2.then_inc(*inc)

            def l1(e):
                i = e % 3
                w = w_sb[e]
                xe = xt[:, e * P : (e + 1) * P]
                if e >= 3:
                    # ph bank of e-3 free once its relus have read it
                    tensor.wait_ge(s_ha, e - 2)
                mm_split(pha[i], w[:, 0:128], xe, True, True, wait=(s_w[e], 16))
                mm_split(
                    phb[i], w[:, 128:256], xe, True, True,
                    wait=(s_hb, e - 2) if e >= 3 else None, inc=(s_ph, 1),
                )

            def l2(e):
                i = e % 2
                w = w_sb[e]
                if e >= 2:
                    # py bank of e-2 free once its y copy completed
                    tensor.wait_ge(s_ye if e % 2 == 0 else s_yo, e // 2)
                mm_split(
                    ppy[i], w[:, 256:384], ha[i][:], True, False, wait=(s_ha, e + 1)
                )
                mm_split(
                    ppy[i], w[:, 384:512], hb[i][:], False, True,
                    wait=(s_hb, e + 1), inc=(s_py, 1),
                )

            # dummy matmuls on scratch data ramp the HAM clock gate to 2.4GHz
            # while the first input DMAs land (~3us of sustained PE activity);
            # they write ppy[0], which l2(0) later resets with start=True
            for _ in range(7):
                nc.tensor.matmul(
                    ppy[0][:, :96], scr[:, :128], scr[:, :96], start=True, stop=True
                )
            tensor.wait_ge(s_x, 16)
            l1(0)
            l1(1)
            tensor.wait_ge(s_xr, 16)
            l1(2)
            for e in range(E):
                if e + 3 <= E - 1:
                    l1(e + 3)
                l2(e)

        # y-copy ops are emitted two experts late so the relu chain (which gates
        # the PE) never queues behind a y op on the same in-order engine.
        def y_scalar(e):
            y = nc.scalar.activation(
                yt[:, e * P : (e + 1) * P],
                ppy[e % 2][:, :P],
                ident,
                bias=w_sb[e][:, 514:515],
            )
            y._wait_ge(s_py, e + 1)
            y.then_inc(s_ye)

        def y_vector(e):
            y = nc.vector.tensor_scalar(
                yt[:, e * P : (e + 1) * P],
                ppy[e % 2][:, :P],
                w_sb[e][:, 514:515],
                None,
                add,
            )
            y._wait_ge(s_py, e + 1)
            y.then_inc(s_yo)

        @block.scalar
        def _(scalar):
            for e in range(E):
                i = e % 2
                if e >= 2:
                    scalar.wait_ge(s_py, e - 1)  # ha buf consumed by e-2's L2
                act = nc.scalar.activation(
                    ha[i][:], pha[e % 3][:, :P], relu, bias=w_sb[e][:, 512:513]
                )
                act._wait_ge(s_ph, e + 1)
                act.then_inc(s_ha)
                if e >= 2 and (e - 2) % 2 == 0:
                    y_scalar(e - 2)
            y_scalar(E - 2)

        @block.vector
        def _(vector):
            for e in range(E):
                i = e % 2
                if e >= 2:
                    vector.wait_ge(s_py, e - 1)
                ts = nc.vector.tensor_scalar(
                    hb[i][:], phb[e % 3][:, :P], w_sb[e][:, 513:514], 0.0, add, amax
                )
                ts._wait_ge(s_ph, e + 1)
                ts.then_inc(s_hb)
                if e >= 2 and (e - 2) % 2 == 1:
                    y_vector(e - 2)
            y_vector(E - 1)

    return nc


def _build_program(P: int) -> bass.Bass:
    nc = bass.Bass("TRN2", target_bir_lowering=False, debug=False, num_devices=N_CORES)
    w_d = nc.declare_dram_parameter(
        "w", [EXP_PER_CORE, 128, WCOLS], F32, isOutput=False
    )
    xt_d = nc.declare_dram_parameter("xt", [128, EXP_PER_CORE * P], F32, isOutput=False)
    yt_d = nc.declare_dram_parameter("yt", [128, EXP_PER_CORE * P], F32, isOutput=True)

    relu = mybir.ActivationFunctionType.Relu
    ident = mybir.ActivationFunctionType.Identity
    add = mybir.AluOpType.add
    amax = mybir.AluOpType.max

    with tile.TileContext(nc) as tc:
        with (
            tc.tile_pool(name="wpool", bufs=8) as wpool,
            tc.tile_pool(name="xpool", bufs=1) as xpool,
            tc.tile_pool(name="hpool", bufs=6) as hpool,
            tc.tile_pool(name="ypool", bufs=1) as ypool,
            tc.tile_pool(name="psum", bufs=2, space="PSUM") as psum,
        ):
            xt = xpool.tile([128, EXP_PER_CORE * P], F32)
            nc.sync.dma_start(xt[:], xt_d[:])
            yt = ypool.tile([128, EXP_PER_CORE * P], F32)

            for e in range(EXP_PER_CORE):
                wt = wpool.tile([128, WCOLS], F32)
                nc.sync.dma_start(wt[:], w_d[e])
                w1t_a = wt[:, 0:128]  # (W1^T)[:, :128]   lhsT for h[:128]
                w1t_b = wt[:, 128:256]  # (W1^T)[:, 128:]   lhsT for h[128:]
                w2t_a = wt[:, 256:384]  # (W2^T)[:128, :]   K = h[:128]
                w2t_b = wt[:, 384:512]  # (W2^T)[128:, :]   K = h[128:]
                b1a = wt[:, 512:513]
                b1b = wt[:, 513:514]
                b2 = wt[:, 514:515]

                if MM_F32R:
                    w1t_a = w1t_a.bitcast(F32R)
                    w1t_b = w1t_b.bitcast(F32R)
                    w2t_a = w2t_a.bitcast(F32R)
                    w2t_b = w2t_b.bitcast(F32R)

                for p0 in range(0, P, 512):
                    pc = min(512, P - p0)
                    xe = xt[:, e * P + p0 : e * P + p0 + pc]
                    if MM_F32R:
                        xe = xe.bitcast(F32R)
                    ph_a = psum.tile([128, pc], F32, tag="ph_a")
                    nc.tensor.matmul(ph_a[:], w1t_a, xe, start=True, stop=True)
                    ph_b = psum.tile([128, pc], F32, tag="ph_b")
                    nc.tensor.matmul(ph_b[:], w1t_b, xe, start=True, stop=True)

                    # bias + relu, split across ACT and DVE to balance engines
                    ha = hpool.tile([128, pc], F32, tag="ha")
                    nc.scalar.activation(ha[:], ph_a[:], relu, bias=b1a)
                    hb = hpool.tile([128, pc], F32, tag="hb")
                    nc.vector.tensor_scalar(hb[:], ph_b[:], b1b, 0.0, add, amax)

                    py = psum.tile([128, pc], F32, tag="py")
                    ha_mm = ha[:].bitcast(F32R) if MM_F32R else ha[:]
                    hb_mm = hb[:].bitcast(F32R) if MM_F32R else hb[:]
                    nc.tensor.matmul(py[:], w2t_a, ha_mm, start=True, stop=False)
                    nc.tensor.matmul(py[:], w2t_b, hb_mm, start=False, stop=True)

                    yo = yt[:, e * P + p0 : e * P + p0 + pc]
                    if e % 2 == 0:
                        nc.scalar.activation(yo, py[:], ident, bias=b2)
                    else:
                        nc.vector.tensor_scalar(yo, py[:], b2, None, add)

            nc.sync.dma_start(yt_d[:], yt[:])
    return nc


def _pack_weights(quant_fn: np.ndarray, first_idx: dict[int, int]) -> np.ndarray:
    wpack = np.zeros((NUM_EMB, 128, WCOLS), dtype=np.float32)
    for e, row in first_idx.items():
        q = quant_fn[row]
        w1 = q[:_I0].reshape(H_SIZE, X_SIZE)  # [256, 128]
        b1 = q[_I0:_I1]  # [256]
        w2 = q[_I1:_I2].reshape(OUT_SIZE, H_SIZE)  # [128, 256]
        b2 = q[_I2:]  # [128]
        wpack[e, :, 0:256] = w1.T
        w2t = np.ascontiguousarray(w2.T)  # [256, 128]
        wpack[e, :, 256:384] = w2t[:128]
        wpack[e, :, 384:512] = w2t[128:]
        wpack[e, :, 512] = b1[:128]
        wpack[e, :, 513] = b1[128:]
        wpack[e, :, 514] = b2
    return wpack


def _run(quant_fn, x, emb_idx, trace=False, tmpdir=None):
    quant_fn = np.asarray(quant_fn, dtype=np.float32)
    x = np.asarray(x, dtype=np.float32)
    idx = np.asarray(emb_idx).astype(np.int64).ravel()
    n = x.shape[0]

    counts = np.bincount(idx, minlength=NUM_EMB)
    order = np.argsort(idx, kind="stable")
    starts = np.zeros(NUM_EMB + 1, dtype=np.int64)
    np.cumsum(counts, out=starts[1:])
    P = int(counts.max())
    P = max((P + 7) // 8 * 8, 8)

    uniq, first = np.unique(idx, return_index=True)
    wpack = _pack_weights(quant_fn, {int(e): int(r) for e, r in zip(uniq, first)})

    xs = x[order]  # [n, 128] grouped by expert
    xt_all = np.zeros((NUM_EMB, 128, P), dtype=np.float32)
    for e in range(NUM_EMB):
        c = counts[e]
        if c:
            xt_all[e, :, :c] = xs[starts[e] : starts[e + 1]].T

    in_maps = []
    for k in range(N_CORES):
        sl = slice(k * EXP_PER_CORE, (k + 1) * EXP_PER_CORE)
        xt_core = np.ascontiguousarray(
            xt_all[sl].transpose(1, 0, 2).reshape(128, EXP_PER_CORE * P)
        )
        in_maps.append({"w": np.ascontiguousarray(wpack[sl]), "xt": xt_core})

    key = (P, RAW_BASS)
    nc = _PROGRAM_CACHE.get(key)
    if nc is None:
        nc = _build_program_raw(P) if RAW_BASS else _build_program(P)
        _PROGRAM_CACHE[key] = nc

    res = run_bass_kernel_spmd(
        nc, in_maps, list(range(N_CORES)), trace=trace, tmpdir=tmpdir
    )

    out = np.empty((n, OUT_SIZE), dtype=np.float32)
    for k in range(N_CORES):
        yt = res.results[k]["yt"]
        for j in range(EXP_PER_CORE):
            e = k * EXP_PER_CORE + j
            c = counts[e]
            if c:
                out[order[starts[e] : starts[e + 1]]] = yt[:, j * P : j * P + c].T
    return out, res


def kernel(**inputs) -> np.ndarray:
    out, _ = _run(inputs["quant_fn"], inputs["x"], inputs["emb_idx"])
    return out


# revision 46
# speedup vs baseline: 1.1031x; 1.0242x over previous
"""MoE-routed batched decoder kernel for Trainium2 (8 NeuronCores, SPMD).

Problem: per-sample 2-layer MLP (128 -> 256 relu -> 128) where each sample's
flat weight vector quant_fn[n] is one of 64 codebook rows selected by
emb_idx[n] (VQ semantics: rows are identical within an expert group).

Strategy (expert-parallel MoE routing):
  host: route tokens by emb_idx (stable sort), dedupe weights to the 64
        unique codebook rows (16.9 MB instead of 1.08 GB), pre-transpose
        into PE lhsT layout, pad each expert's token group to P columns;
  device (SPMD x8): core k owns experts [8k, 8k+8); per expert two
        fp32 matmuls for layer 1 (W1^T halves), fused bias+relu, two
        accumulating matmuls for layer 2 (W2^T halves), fused bias add;
  host: inverse-permute per-expert outputs back to token order.
"""

import sys
import types

import numpy as np

for _p in ("/opt/trn_rl_repo", "/root/.axon_site/_ro/trn_rl_repo"):
    if _p not in sys.path:
        sys.path.append(_p)

import concourse.bass as bass
import concourse.tile as tile
from concourse import mybir
from concourse.bass_utils import run_bass_kernel_spmd
from bass_rust import ScopedClock

X_SIZE = 128
H_SIZE = 256
OUT_SIZE = 128
NUM_EMB = 64
N_CORES = 8
EXP_PER_CORE = NUM_EMB // N_CORES
# packed per-expert weight block columns: W1^T [128,256] | W2^T upper [128,128]
# | W2^T lower [128,128] | b1[:128] | b1[128:] | b2 | pad
WCOLS = 516
_I0 = H_SIZE * X_SIZE
_I1 = _I0 + H_SIZE
_I2 = _I1 + OUT_SIZE * H_SIZE

F32 = mybir.dt.float32
F32R = mybir.dt.float32r
MM_F32R = False  # float32r needs explicit rounding ops and loses precision; keep fp32


def _install_axon_shims():
    """Make trace=True survivable in this container: register the ctypes NTFF
    hook for the missing antenv.axon_hooks module, and keep profile artifacts
    local (no bucket creds in the sandbox)."""
    try:
        import antenv

        if "antenv.axon_hooks" not in sys.modules:
            shim = types.ModuleType("antenv.axon_hooks")
            holder = {}
            shim.set_axon_ntff_profile_hook = lambda h: holder.__setitem__("h", h)
            shim.get_axon_ntff_profile_hook = lambda: holder.get("h")
            sys.modules["antenv.axon_hooks"] = shim
            antenv.axon_hooks = shim
            import trn_agent_boot.trn_boot as _tb

            shim.set_axon_ntff_profile_hook(
                _tb._ntff_profile_via_ctypes("/opt/axon/libaxon_pjrt.so")
            )
        import concourse.bass_utils as _bu

        _bu.upload_artifacts = lambda tmpdir: "local://" + tmpdir
    except Exception:
        pass


def _patched_drain_and_barrier(self, tick_clock, wait_clock):
    # The stock implementation piles every outstanding sem wait onto the single
    # tail Drain CTRL, which overflows this walrus build's sync-wait slots.
    # Emit one wait_ge per semaphore on the sync engine, then a bare drain.
    probe = mybir.InstNoOp(
        name="tile_drain_probe", engine=mybir.EngineType.SP, bass_nofuse=True
    )
    wait_clock.add_sem_waits(probe, ScopedClock({None: tick_clock.global_clock}))
    by_name = {h.name: h for h in self.sems.allocated().values()}
    for w in probe.sync_info.on_wait if probe.sync_info else []:
        self.nc.sync.wait_ge(by_name[w.ant_name], w.wait_value)
    self.nc.sync.drain()
    self.nc.all_engine_barrier()
    popped = self.nc._tile_sem_poison_stack.pop()
    assert popped is self._sem_poison
    self.nc.clear_and_free_semaphores(list(self.sems.allocated().values()))
    self.nc.all_engine_barrier()


_orig_lower_ordered_insts = tile.TileContext._lower_ordered_insts
_waitsplit_counter = [0]


def _lower_with_wait_split(self, ordered):
    # This walrus build rejects instructions carrying more than one sync wait
    # ("Too many sync wait commands"). Hoist excess waits into dedicated
    # single-wait NoOps on the same engine, immediately before the offender.
    for bb_name, insts in ordered.items():
        new = []
        for inst in insts:
            si = inst.sync_info
            if si is not None and len(si.on_wait) > 1:
                waits = list(si.on_wait)
                for w in waits[:-1]:
                    _waitsplit_counter[0] += 1
                    new.append(
                        mybir.InstNoOp(
                            name=f"I-waitsplit-{_waitsplit_counter[0]}",
                            engine=inst.engine,
                            sync_info=mybir.SyncInfo(on_wait=[w], on_update=[]),
                            bass_nofuse=True,
                        )
                    )
                inst.sync_info = mybir.SyncInfo(
                    on_wait=[waits[-1]], on_update=list(si.on_update)
                )
            new.append(inst)
        ordered[bb_name] = new
    return _orig_lower_ordered_insts(self, ordered)


tile.TileContext._lower_ordered_insts = _lower_with_wait_split
tile.TileContext._drain_and_barrier = _patched_drain_and_barrier
_install_axon_shims()

_PROGRAM_CACHE: dict[tuple, bass.Bass] = {}
RAW_BASS = True  # hand-scheduled pipeline (no TileContext) — much smaller head/tail
LEAN_CTOR = True  # strip Bass-ctor preamble/barrier (engine-register init unused here)


class _LeanBass(bass.Bass):
    """Bass that skips the ctor-emitted engine preambles and the post-const
    all-engine barrier. Our kernel does its own cross-engine sync from scratch
    semaphores, and the preamble register inits aren't used by the instruction
    mix here (DMA / matmul / activation / tensor_scalar)."""

    def all_engine_barrier(self, **kw):
        if getattr(self, "_lean_done", False):
            return super().all_engine_barrier(**kw)


def _make_bass() -> bass.Bass:
    if not LEAN_CTOR:
        return bass.Bass(
            "TRN2", target_bir_lowering=False, debug=False, num_devices=N_CORES
        )
    orig_preamble = bass.BassEngine.preamble
    bass.BassEngine.preamble = lambda self: None
    try:
        nc = _LeanBass(
            "TRN2", target_bir_lowering=False, debug=False, num_devices=N_CORES
        )
    finally:
        bass.BassEngine.preamble = orig_preamble
    nc._lean_done = True
    return nc


def _build_program_raw(P: int) -> bass.Bass:
    """Static 4-engine pipeline with manual semaphores.

    sync   : x DMA, per-expert weight DMA in, per-expert y DMA out
    tensor : per expert, 4 fp32 matmuls (2x L1 halves, 2x accumulating L2)
    scalar : relu+bias for the first H half; y bias+copy for even experts
    vector : relu+bias for the second H half; y bias+copy for odd experts
    PSUM   : ph_a/ph_b/py double-buffered by expert parity (6 banks)
    """
    E = EXP_PER_CORE
    nc = _make_bass()
    w_d = nc.declare_dram_parameter("w", [E, 128, WCOLS], F32, isOutput=False)
    xt_d = nc.declare_dram_parameter("xt", [128, E * P], F32, isOutput=False)
    yt_d = nc.declare_dram_parameter("yt", [128, E * P], F32, isOutput=True)

    relu = mybir.ActivationFunctionType.Relu
    ident = mybir.ActivationFunctionType.Identity
    add = mybir.AluOpType.add
    amax = mybir.AluOpType.max

    xt = nc.alloc_sbuf_tensor("xt_sb", [128, E * P], F32).ap()
    yt = nc.alloc_sbuf_tensor("yt_sb", [128, E * P], F32).ap()
    w_all = nc.alloc_sbuf_tensor("w_sb", [128, E * WCOLS], F32).ap()
    w_sb = [w_all[:, e * WCOLS : (e + 1) * WCOLS] for e in range(E)]
    scr = nc.alloc_sbuf_tensor("scratch", [128, 512], F32).ap()
    ha = [nc.alloc_sbuf_tensor(f"ha{i}", [128, P], F32).ap() for i in range(2)]
    hb = [nc.alloc_sbuf_tensor(f"hb{i}", [128, P], F32).ap() for i in range(2)]
    pha = [nc.alloc_psum_tensor(f"pha{i}", [128, 512], F32).ap() for i in range(3)]
    phb = [nc.alloc_psum_tensor(f"phb{i}", [128, 512], F32).ap() for i in range(3)]
    ppy = [nc.alloc_psum_tensor(f"ppy{i}", [128, 512], F32).ap() for i in range(2)]
    # DRAM weights viewed [partition, expert, col] for paired-expert DMAs
    w_dp = w_d.ap().rearrange("e p c -> p e c")

    from contextlib import ExitStack

    with ExitStack() as st:
        sem = lambda name: st.enter_context(nc.semaphore(name))
        s_x = sem("s_x")
        s_xr = sem("s_xr")
        s_w = [sem(f"s_w{e}") for e in range(E)]
        s_ph = sem("s_ph")
        s_ha, s_hb = sem("s_ha"), sem("s_hb")
        s_py = sem("s_py")
        s_ye, s_yo = sem("s_ye"), sem("s_yo")
        s_out = sem("s_out")
        block = st.enter_context(nc.Block())

        @block.sync
        def _(sync):
            # weights in expert order on the sync HWDGE queue; x remainder and
            # output DMAs ride the otherwise-idle gpsimd queue in parallel
            sync.dma_start(out=xt[:, : 2 * P], in_=xt_d[:, : 2 * P]).then_inc(s_x, 16)
            for e in range(E):
                sync.dma_start(
                    out=w_sb[e][:], in_=w_dp[:, e, :]
                ).then_inc(s_w[e], 16)

        @block.gpsimd
        def _(gpsimd):
            gpsimd.dma_start(out=xt[:, 2 * P :], in_=xt_d[:, 2 * P :]).then_inc(
                s_xr, 16
            )
            for j in range(E // 2):
                # two experts' outputs per DMA (even parity fused, odd standalone)
                gpsimd.wait_ge(s_yo, j + 1)
                dma = gpsimd.dma_start(
                    out=yt_d[:, 2 * j * P : (2 * j + 2) * P],
                    in_=yt[:, 2 * j * P : (2 * j + 2) * P],
                )
                dma._wait_ge(s_ye, j + 1)
                dma.then_inc(s_out, 16)
            gpsimd.wait_ge(s_out, 16 * (E // 2))

        @block.tensor
        def _(tensor):
            # Software-pipelined: L1 of expert e+1 issues before L2 of expert e,
            # so the relu handoff (PE -> ACT/DVE -> PE) hides behind L1 matmuls.
            # Each logical matmul is col-split into two concurrent M=64 tiles
            # (tile_position (0,0)/(0,64)): the 64-col LDWEIGHTS pair overlaps,
            # halving the weight-load bound that dominates fp32 passes.
            NSPLIT = 4  # concurrent column tiles per logical matmul
            MW = 128 // NSPLIT

            def mm_split(out, lhsT, rhs, start, stop, wait=None, inc=None):
                for j in range(NSPLIT):
                    m = nc.tensor.matmul(
                        out[j * MW : (j + 1) * MW, :P],
                        lhsT[:, j * MW : (j + 1) * MW],
                        rhs,
                        start=start, stop=stop, tile_position=(0, j * MW),
                    )
                    if j == 0 and wait is not None:
                        m._wait_ge(*wait)
                    if j == NSPLIT - 1 and inc is not None:
                        m.then_inc(*inc)

            def l1(e):
                i = e % 3
                w = w_sb[e]
                xe = xt[:, e * P : (e + 1) * P]
                if e >= 3:
                    # ph bank of e-3 free once its relus have read it
                    tensor.wait_ge(s_ha, e - 2)
                mm_split(pha[i], w[:, 0:128], xe, True, True, wait=(s_w[e], 16))
                mm_split(
                    phb[i], w[:, 128:256], xe, True, True,
                    wait=(s_hb, e - 2) if e >= 3 else None, inc=(s_ph, 1),
                )

            def l2(e):
                i = e % 2
                w = w_sb[e]
                if e >= 2:
                    # py bank of e-2 free once its y copy completed
                    tensor.wait_ge(s_ye if e % 2 == 0 else s_yo, e // 2)
                mm_split(
                    ppy[i], w[:, 256:384], ha[i][:], True, False, wait=(s_ha, e + 1)
                )
                mm_split(
                    ppy[i], w[:, 384:512], hb[i][:], False, True,
                    wait=(s_hb, e + 1), inc=(s_py, 1),
                )

            # dummy matmuls on scratch data ramp the HAM clock gate to 2.4GHz
            # while the first input DMAs land (~3us of sustained PE activity);
            # they write ppy[0], which l2(0) later resets with start=True
            for _ in range(7):
                nc.tensor.matmul(
                    ppy[0][:, :96], scr[:, :128], scr[:, :96], start=True, stop=True
                )
            tensor.wait_ge(s_x, 16)
            l1(0)
            l1(1)
            tensor.wait_ge(s_xr, 16)
            l1(2)
            for e in range(E):
                if e + 3 <= E - 1:
                    l1(e + 3)
                l2(e)

        # y-copy ops are emitted two experts late so the relu chain (which gates
        # the PE) never queues behind a y op on the same in-order engine.
        def y_scalar(e):
            y = nc.scalar.activation(
                yt[:, e * P : (e + 1) * P],
                ppy[e % 2][:, :P],
                ident,
                bias=w_sb[e][:, 514:515],
            )
            y._wait_ge(s_py, e + 1)
            y.then_inc(s_ye)

        def y_vector(e):
            y = nc.vector.tensor_scalar(
                yt[:, e * P : (e + 1) * P],
                ppy[e % 2][:, :P],
                w_sb[e][:, 514:515],
                None,
                add,
            )
            y._wait_ge(s_py, e + 1)
            y.then_inc(s_yo)

        @block.scalar
        def _(scalar):
            for e in range(E):
                i = e % 2
                if e >= 2:
                    scalar.wait_ge(s_py, e - 1)  # ha buf consumed by e-2's L2
                act = nc.scalar.activation(
                    ha[i][:], pha[e % 3][:, :P], relu, bias=w_sb[e][:, 512:513]
                )
                act._wait_ge(s_ph, e + 1)
                act.then_inc(s_ha)
                if e >= 2 and (e - 2) % 2 == 0:
                    y_scalar(e - 2)
            y_scalar(E - 2)

        @block.vector
        def _(vector):
            for e in range(E):
                i = e % 2
                if e >= 2:
                    vector.wait_ge(s_py, e - 1)
                ts = nc.vector.tensor_scalar(
                    hb[i][:], phb[e % 3][:, :P], w_sb[e][:, 513:514], 0.0, add, amax
                )
                ts._wait_ge(s_ph, e + 1)
                ts.then_inc(s_hb)
                if e >= 2 and (e - 2) % 2 == 1:
                    y_vector(e - 2)
            y_vector(E - 1)

    return nc


def _build_program(P: int) -> bass.Bass:
    nc = bass.Bass("TRN2", target_bir_lowering=False, debug=False, num_devices=N_CORES)
    w_d = nc.declare_dram_parameter(
        "w", [EXP_PER_CORE, 128, WCOLS], F32, isOutput=False
    )
    xt_d = nc.declare_dram_parameter("xt", [128, EXP_PER_CORE * P], F32, isOutput=False)
    yt_d = nc.declare_dram_parameter("yt", [128, EXP_PER_CORE * P], F32, isOutput=True)

    relu = mybir.ActivationFunctionType.Relu
    ident = mybir.ActivationFunctionType.Identity
    add = mybir.AluOpType.add
    amax = mybir.AluOpType.max

    with tile.TileContext(nc) as tc:
        with (
            tc.tile_pool(name="wpool", bufs=8) as wpool,
            tc.tile_pool(name="xpool", bufs=1) as xpool,
            tc.tile_pool(name="hpool", bufs=6) as hpool,
            tc.tile_pool(name="ypool", bufs=1) as ypool,
            tc.tile_pool(name="psum", bufs=2, space="PSUM") as psum,
        ):
            xt = xpool.tile([128, EXP_PER_CORE * P], F32)
            nc.sync.dma_start(xt[:], xt_d[:])
            yt = ypool.tile([128, EXP_PER_CORE * P], F32)

            for e in range(EXP_PER_CORE):
                wt = wpool.tile([128, WCOLS], F32)
                nc.sync.dma_start(wt[:], w_d[e])
                w1t_a = wt[:, 0:128]  # (W1^T)[:, :128]   lhsT for h[:128]
                w1t_b = wt[:, 128:256]  # (W1^T)[:, 128:]   lhsT for h[128:]
                w2t_a = wt[:, 256:384]  # (W2^T)[:128, :]   K = h[:128]
                w2t_b = wt[:, 384:512]  # (W2^T)[128:, :]   K = h[128:]
                b1a = wt[:, 512:513]
                b1b = wt[:, 513:514]
                b2 = wt[:, 514:515]

                if MM_F32R:
                    w1t_a = w1t_a.bitcast(F32R)
                    w1t_b = w1t_b.bitcast(F32R)
                    w2t_a = w2t_a.bitcast(F32R)
                    w2t_b = w2t_b.bitcast(F32R)

                for p0 in range(0, P, 512):
                    pc = min(512, P - p0)
                    xe = xt[:, e * P + p0 : e * P + p0 + pc]
                    if MM_F32R:
                        xe = xe.bitcast(F32R)
                    ph_a = psum.tile([128, pc], F32, tag="ph_a")
                    nc.tensor.matmul(ph_a[:], w1t_a, xe, start=True, stop=True)
                    ph_b = psum.tile([128, pc], F32, tag="ph_b")
                    nc.tensor.matmul(ph_b[:], w1t_b, xe, start=True, stop=True)

                    # bias + relu, split across ACT and DVE to balance engines
                    ha = hpool.tile([128, pc], F32, tag="ha")
                    nc.scalar.activation(ha[:], ph_a[:], relu, bias=b1a)
                    hb = hpool.tile([128, pc], F32, tag="hb")
                    nc.vector.tensor_scalar(hb[:], ph_b[:], b1b, 0.0, add, amax)

                    py = psum.tile([128, pc], F32, tag="py")
                    ha_mm = ha[:].bitcast(F32R) if MM_F32R else ha[:]
                    hb_mm = hb[:].bitcast(F32R) if MM_F32R else hb[:]
                    nc.tensor.matmul(py[:], w2t_a, ha_mm, start=True, stop=False)
                    nc.tensor.matmul(py[:], w2t_b, hb_mm, start=False, stop=True)

                    yo = yt[:, e * P + p0 : e * P + p0 + pc]
                    if e % 2 == 0:
                        nc.scalar.activation(yo, py[:], ident, bias=b2)
                    else:
                        nc.vector.tensor_scalar(yo, py[:], b2, None, add)

            nc.sync.dma_start(yt_d[:], yt[:])
    return nc


def _pack_weights(quant_fn: np.ndarray, first_idx: dict[int, int]) -> np.ndarray:
    wpack = np.zeros((NUM_EMB, 128, WCOLS), dtype=np.float32)
    for e, row in first_idx.items():
        q = quant_fn[row]
        w1 = q[:_I0].reshape(H_SIZE, X_SIZE)  # [256, 128]
        b1 = q[_I0:_I1]  # [256]
        w2 = q[_I1:_I2].reshape(OUT_SIZE, H_SIZE)  # [128, 256]
        b2 = q[_I2:]  # [128]
        wpack[e, :, 0:256] = w1.T
        w2t = np.ascontiguousarray(w2.T)  # [256, 128]
        wpack[e, :, 256:384] = w2t[:128]
        wpack[e, :, 384:512] = w2t[128:]
        wpack[e, :, 512] = b1[:128]
        wpack[e, :, 513] = b1[128:]
        wpack[e, :, 514] = b2
    return wpack


def _run(quant_fn, x, emb_idx, trace=False, tmpdir=None):
    quant_fn = np.asarray(quant_fn, dtype=np.float32)
    x = np.asarray(x, dtype=np.float32)
    idx = np.asarray(emb_idx).astype(np.int64).ravel()
    n = x.shape[0]

    counts = np.bincount(idx, minlength=NUM_EMB)
    order = np.argsort(idx, kind="stable")
    starts = np.zeros(NUM_EMB + 1, dtype=np.int64)
    np.cumsum(counts, out=starts[1:])
    P = int(counts.max())
    P = max((P + 7) // 8 * 8, 8)

    uniq, first = np.unique(idx, return_index=True)
    wpack = _pack_weights(quant_fn, {int(e): int(r) for e, r in zip(uniq, first)})

    xs = x[order]  # [n, 128] grouped by expert
    xt_all = np.zeros((NUM_EMB, 128, P), dtype=np.float32)
    for e in range(NUM_EMB):
        c = counts[e]
        if c:
            xt_all[e, :, :c] = xs[starts[e] : starts[e + 1]].T

    in_maps = []
    for k in range(N_CORES):
        sl = slice(k * EXP_PER_CORE, (k + 1) * EXP_PER_CORE)
        xt_core = np.ascontiguousarray(
            xt_all[sl].transpose(1, 0, 2).reshape(128, EXP_PER_CORE * P)
        )
        in_maps.append({"w": np.ascontiguousarray(wpack[sl]), "xt": xt_core})

    key = (P, RAW_BASS)
    nc = _PROGRAM_CACHE.get(key)
    if nc is None:
        nc = _build_program_raw(P) if RAW_BASS else _build_program(P)
        _PROGRAM_CACHE[key] = nc

    res = run_bass_kernel_spmd(
        nc, in_maps, list(range(N_CORES)), trace=trace, tmpdir=tmpdir
    )

    out = np.empty((n, OUT_SIZE), dtype=np.float32)
    for k in range(N_CORES):
        yt = res.results[k]["yt"]
        for j in range(EXP_PER_CORE):
            e = k * EXP_PER_CORE + j
            c = counts[e]
            if c:
                out[order[starts[e] : starts[e + 1]]] = yt[:, j * P : j * P + c].T
    return out, res


def kernel(**inputs) -> np.ndarray:
    out, _ = _run(inputs["quant_fn"], inputs["x"], inputs["emb_idx"])
    return out
